# revision 32
# baseline (speedup 1.0000x reference)
"""NT-Xent contrastive loss kernel for 8 Trainium2 NeuronCores.

Reference computation (N=8192, D=512, tau=0.5):
    zl = l2norm_rows(left); zr = l2norm_rows(right)
    refl    = exp(zl @ zl.T / tau)
    between = exp(zl @ zr.T / tau)
    denom   = refl.sum(1) + between.sum(1) - diag(refl)
    loss    = -log(diag(between) / denom)

Fused per-row form (diag(refl) == e^2 exactly since rows of zl are unit):
    loss[m] = log( S_l[m] + S_r[m] - e^2 ) - 2 * (zl_m . zr_m)
with S_x[m] = sum_n exp(2 * zl_m . zx_n).

Sharding: data-parallel over rows; core c owns rows [c*1024, (c+1)*1024).
The host ships, per core, both tensors in a column-ROLLED chunked K-major
fp8e4m3 layout: columns are rotated so the core's own 1024 columns come
first, then split into 4 chunks of 2048 columns shaped [128, 4, 2048]
(partition = k%128, then k//128, then column); each chunk is 1MB with
16KB contiguous per-partition lines.  Input DMAs round-robin over all
three DGE queues (SP + ACT HWDGE, Pool SWDGE) — a single dynamic queue
is serviced by one ~23GB/s DMA engine, which was the baseline kernel's
actual bottleneck.

Per chunk the kernel streams: square (DVE) -> per-column sum-of-squares
via ones-matmul into a [1,2048] PSUM strip (PE) -> inv-norm =
Exp(-0.5*Ln(ssq)) (ACT; the compile patch below pins the one act table
containing BOTH ln and exp so no table reloads occur) ->
partition_broadcast (GpSimd) -> normalize-multiply into fp8e4 (DVE).
Chunk 0 (the lhsT source) is DMA'd column-sliced and normalized in
512-column pieces so the pipeline fills early; its normalized output is
the persistent lhsT.  Main similarity matmuls run in fp8 DoubleRow mode
(K=256 per instruction, 0.5 cycles/row) into [128,2048] PSUM tiles; one
exp activation per (m-tile, chunk) consumes the tile in place with
accum_out producing the partial row-sum.  The next chunk's norm chain is
emitted mid-m-loop so the in-order PE never queues it ahead of main
matmuls.  The between-diagonal is masked off the exp'd own-r PSUM and
log-subtracted in the epilogue.  No cross-core communication.
"""

import numpy as np
import ml_dtypes
from contextlib import ExitStack

import concourse.bass as bass
import concourse.tile as tile
from concourse import bacc, mybir
from concourse.bass import ds, ts
from concourse.bass_utils import run_bass_kernel_spmd
from concourse.masks import make_identity

P = 128          # partitions
D = 512          # feature dim
N = 8192         # rows
NCORES = 8
BLK = N // NCORES          # 1024 rows per core
KCH = D // P               # 4 k-chunks of 128
MT = BLK // P              # 8 m-tiles per core
W = 2048                   # columns per streamed chunk
NJ = N // W                # 4 chunks per tensor
NCHUNK = 2 * NJ            # 8 chunks total (l then r)
NG = W // 512              # 4 psum col-groups per chunk
E2 = float(np.exp(2.0))

F32 = mybir.dt.float32
BF16 = mybir.dt.bfloat16
FP8 = mybir.dt.float8e4
AF = mybir.ActivationFunctionType
OP = mybir.AluOpType
DR = mybir.MatmulPerfMode.DoubleRow

_CACHE = {}


def _body(ctx, tc, lch, rch, loss_out):
    nc = tc.nc

    const_pool = ctx.enter_context(tc.tile_pool(name="const", bufs=1))
    persist = ctx.enter_context(tc.tile_pool(name="persist", bufs=1))
    raw_pool = ctx.enter_context(tc.tile_pool(name="raw", bufs=4))
    sq_pool = ctx.enter_context(tc.tile_pool(name="sq", bufs=2))
    nrm_pool = ctx.enter_context(tc.tile_pool(name="nrm", bufs=2))
    zn_pool = ctx.enter_context(tc.tile_pool(name="zn", bufs=3))
    small = ctx.enter_context(tc.tile_pool(name="small", bufs=2))

    psum = ctx.enter_context(tc.tile_pool(name="ps", bufs=2, space="PSUM"))

    # constants
    ones_col = const_pool.tile([P, 1], BF16, tag="ones_col")
    nc.gpsimd.memset(ones_col[:], 1.0)
    ident = const_pool.tile([P, P], F32, tag="ident")
    make_identity(nc, ident[:])
    neg_e2 = const_pool.tile([P, 1], F32, tag="neg_e2")
    nc.gpsimd.memset(neg_e2[:], -E2)

    # persistent tensors
    zn_own = persist.tile([P, KCH, W], FP8, tag="zn_own")   # chunk 0 of l
    rowsums = persist.tile([P, MT, NCHUNK], F32, tag="rowsums")
    bd = persist.tile([P, MT], F32, tag="bd")
    diags = persist.tile([P, MT, P], F32, tag="diags")
    m0x = persist.tile([P, NG - 1], F32, tag="m0x")

    raws = {}
    # Round-robin the input DMAs over all three DGE queues (SP and ACT
    # HWDGE + Pool SWDGE): each dynamic queue is serviced by a single DMA
    # engine at ~23GB/s, so a single queue serializes the whole input
    # stream (this was the baseline's actual bottleneck).
    dma_engines = [nc.sync, nc.scalar, nc.gpsimd]

    def dma_stage(j):
        src = lch[j, :, :, :] if j < NJ else rch[j - NJ, :, :, :]
        raw = raw_pool.tile([P, KCH, W], FP8, tag="raw")
        if j == 0:
            # chunk 0 gates the whole pipeline (it is the lhsT source):
            # column-sliced piece DMAs (512 cols = 512B descriptor lines,
            # the minimum for full DMA-bus efficiency) land piece 0 after
            # one slice time so the piecewise norm chain starts early.
            for g in range(NG):
                dma_engines[g % 3].dma_start(
                    raw[:, :, ds(g * 512, 512)], src[:, :, ds(g * 512, 512)]
                )
        else:
            for k in range(KCH):
                dma_engines[(j * KCH + k) % 3].dma_start(raw[:, k, :], src[:, k, :])
        raws[j] = raw

    zns = {}

    def norm_stage(j):
        raw = raws.pop(j)
        sq = sq_pool.tile([P, KCH, W], BF16, tag="sq")
        # chunk 0 is processed in 512-column pieces (its DMA is
        # column-sliced), everything else in one pass
        pieces = NG if j == 0 else 1
        pw = W // pieces
        # Shares the "act" ring slots (same tag) so PSUM stays within 8
        # banks; ring order interleaves [ssq_j+1, act_j_m0..m7, ...].
        ssq = psum.tile([1, W], F32, tag="act")
        lssq = nrm_pool.tile([1, W], F32, tag="lssq")
        inv1 = nrm_pool.tile([1, W], BF16, tag="inv1")
        invb = nrm_pool.tile([P, W], BF16, tag="invb")
        zn = zn_own if j == 0 else zn_pool.tile([P, KCH, W], FP8, tag="zn")
        for p_ in range(pieces):
            cs = ds(p_ * pw, pw)
            nc.vector.tensor_mul(sq[:, :, cs], raw[:, :, cs], raw[:, :, cs])
            for g in range(pw // 512):
                c0 = p_ * pw + g * 512
                for k in range(KCH):
                    nc.tensor.matmul(
                        ssq[:, ds(c0, 512)],
                        ones_col[:],
                        sq[:, k, ds(c0, 512)],
                        start=(k == 0),
                        stop=(k == KCH - 1),
                    )
            nc.scalar.activation(lssq[:, cs], ssq[:, cs], AF.Ln)
            nc.scalar.activation(inv1[:, cs], lssq[:, cs], AF.Exp, scale=-0.5)
            nc.gpsimd.partition_broadcast(invb[:, cs], inv1[:, cs])
            for k in range(KCH):
                nc.vector.tensor_mul(zn[:, k, cs], raw[:, k, cs], invb[:, cs])
        zns[j] = zn

    def main_stage(j):
        zn = zns.pop(j)
        for m in range(MT):
            ps = psum.tile([P, W], F32, tag="act")
            split = j == 0 and m == 0
            for g in range(NG):
                for i in range(KCH // 2):
                    nc.tensor.matmul(
                        ps[:, ds(g * 512, 512)],
                        zn_own[:, ds(2 * i, 2), ts(m, P)],
                        zn[:, ds(2 * i, 2), ds(g * 512, 512)],
                        start=(i == 0),
                        stop=(i == KCH // 2 - 1),
                        perf_mode=DR,
                    )
                if split:
                    # very first tile: exp per 512-col group so the first
                    # activation fires as soon as chunk-0 piece g is in
                    nc.scalar.activation(
                        ps[:, ds(g * 512, 512)], ps[:, ds(g * 512, 512)],
                        AF.Exp, scale=2.0,
                        accum_out=rowsums[:, 0, ds(0, 1)] if g == 0
                        else m0x[:, ds(g - 1, 1)],
                    )
            # exp output is discarded (only accum_out matters): write back
            # into the same PSUM tile (PSUM access is cheaper for ACT than
            # SBUF, and no SBUF bandwidth is spent on a dummy tensor).
            if not split:
                nc.scalar.activation(
                    ps[:], ps[:], AF.Exp, scale=2.0,
                    accum_out=rowsums[:, m, ds(j, 1)],
                )
            if j == NJ:
                # own-r chunk: the exp'd diagonal is exp(2*zl_m.zr_m); mask
                # it out now (one cheap DVE op frees the PSUM slot), reduce
                # and Ln it in the epilogue off the critical path.
                nc.vector.tensor_mul(
                    diags[:, m, :], ps[:, ds(m * P, P)], ident[:]
                )
            if m == 1:
                # software pipeline: the next chunk's DMA + norm chain is
                # emitted here, AFTER this chunk's first two m-tiles, so
                # the in-order PE never stalls main matmuls behind the
                # next chunk's ssq (whose inputs may still be in flight).
                if j + 3 < NCHUNK:
                    dma_stage(j + 3)
                if j + 1 < NCHUNK:
                    norm_stage(j + 1)

    dma_stage(0)
    dma_stage(1)
    dma_stage(2)
    norm_stage(0)
    for j in range(NCHUNK):
        main_stage(j)

    # ---- loss epilogue -----------------------------------------------------
    # loss = Ln(S_l + S_r - e^2) - Ln(exp(2*zl.zr))  [bd holds the exp'd diag]
    s_all = small.tile([P, MT], F32, tag="s_all")
    nc.vector.tensor_reduce(
        s_all[:], rowsums[:], axis=mybir.AxisListType.X, op=OP.add
    )
    m0s = small.tile([P, 1], F32, tag="m0s")
    nc.vector.tensor_reduce(
        m0s[:], m0x[:], axis=mybir.AxisListType.X, op=OP.add
    )
    nc.vector.tensor_add(s_all[:, ds(0, 1)], s_all[:, ds(0, 1)], m0s[:])
    logd = small.tile([P, MT], F32, tag="logd")
    nc.scalar.activation(logd[:], s_all[:], AF.Ln, bias=neg_e2[:])
    nc.vector.tensor_reduce(
        bd[:], diags[:], axis=mybir.AxisListType.X, op=OP.add
    )
    logdd = small.tile([P, MT], F32, tag="logdd")
    nc.scalar.activation(logdd[:], bd[:], AF.Ln)
    loss_sb = small.tile([P, MT], F32, tag="loss_sb")
    nc.vector.tensor_sub(loss_sb[:], logd[:], logdd[:])
    nc.sync.dma_start(loss_out[:, :], loss_sb[:])


class _pin_act_table:
    """During compile, present activation tables where Exp/Ln appear ONLY in
    the combined natural_log_exp table, so the table-load pass emits a single
    hoisted load instead of ping-ponging between an exp-table and an
    ln-table (1.3us per switch).  Table order/indices are preserved; only
    the membership sets are filtered.  Restored immediately after compile."""

    COMBINED = "natural_log_exp_and_others"

    def __enter__(self):
        import concourse.bacc as bacc_mod
        self._mod = bacc_mod
        self._orig = bacc_mod.get_activation_tables

        orig = self._orig
        combined = self.COMBINED

        def patched(arch):
            tabs = orig(arch)
            if combined not in tabs:
                return tabs
            pin = {AF.Exp, AF.Ln}
            out = {}
            for name, s in tabs.items():
                out[name] = set(s) if name == combined else set(s) - pin
            return out

        bacc_mod.get_activation_tables = patched
        return self

    def __exit__(self, *exc):
        self._mod.get_activation_tables = self._orig
        return False


def _build():
    nc = bacc.Bacc("TRN2", target_bir_lowering=False, debug=False, num_devices=NCORES)
    lch = nc.dram_tensor("lch", [NJ, P, KCH, W], FP8, kind="ExternalInput").ap()
    rch = nc.dram_tensor("rch", [NJ, P, KCH, W], FP8, kind="ExternalInput").ap()
    loss = nc.dram_tensor("loss", [P, MT], F32, kind="ExternalOutput").ap()
    with tile.TileContext(nc) as tc, ExitStack() as ctx:
        _body(ctx, tc, lch, rch, loss)
    with _pin_act_table():
        nc.compile()
    return nc


def _get_nc():
    if "nc" not in _CACHE:
        _CACHE["nc"] = _build()
    return _CACHE["nc"]


def _chunked(xT, c):
    """xT: [KCH, P, N] bf16 K-major. Returns [NJ, P, KCH, W] rolled so core
    c's own columns come first."""
    r = np.roll(xT, -c * BLK, axis=2)
    # [KCH, P, N] -> [NJ, P, KCH, W]
    out = np.empty((NJ, P, KCH, W), dtype=xT.dtype)
    for j in range(NJ):
        out[j] = r[:, :, j * W:(j + 1) * W].transpose(1, 0, 2)
    return out


def _in_maps(left, right):
    f8 = ml_dtypes.float8_e4m3
    left = np.asarray(left, dtype=np.float32)
    right = np.asarray(right, dtype=np.float32)
    lT = np.ascontiguousarray(left.T).astype(f8).reshape(KCH, P, N)
    rT = np.ascontiguousarray(right.T).astype(f8).reshape(KCH, P, N)
    maps = []
    for c in range(NCORES):
        maps.append({
            "lch": _chunked(lT, c),
            "rch": _chunked(rT, c),
        })
    return maps


def _gather(results):
    # loss dram tile is [128 partitions, 8 m-tiles]; row m = t*128 + p
    parts = [np.asarray(r["loss"]).T.reshape(-1) for r in results]
    return np.concatenate(parts).astype(np.float32)


def run_traced(left, right):
    """Run with NTFF profiling; returns (loss, exec_time_ns)."""
    res = run_bass_kernel_spmd(
        _get_nc(), _in_maps(left, right), list(range(NCORES)), trace=True
    )
    return _gather(res.results), res.exec_time_ns


def kernel(left, right):
    res = run_bass_kernel_spmd(
        _get_nc(), _in_maps(left, right), list(range(NCORES))
    )
    return _gather(res.results)


# revision 43
# speedup vs baseline: 1.3474x; 1.3474x over previous
"""NT-Xent contrastive loss kernel for 8 Trainium2 NeuronCores.

Reference computation (N=8192, D=512, tau=0.5):
    zl = l2norm_rows(left); zr = l2norm_rows(right)
    refl    = exp(zl @ zl.T / tau)
    between = exp(zl @ zr.T / tau)
    denom   = refl.sum(1) + between.sum(1) - diag(refl)
    loss    = -log(diag(between) / denom)

Fused per-row form (diag(refl) == e^2 exactly since rows of zl are unit):
    loss[m] = log( S_l[m] + S_r[m] - e^2 ) - 2 * (zl_m . zr_m)
with S_x[m] = sum_n exp(2 * zl_m . zx_n).

Sharding: data-parallel over rows; core c owns rows [c*1024, (c+1)*1024).
The host ships, per core, both tensors in a column-ROLLED chunked K-major
fp8e4m3 layout: columns are rotated so the core's own 1024 columns come
first, then split into 4 chunks of 2048 columns shaped [128, 4, 2048]
(partition = k%128, then k//128, then column); each chunk is 1MB with
16KB contiguous per-partition lines.  Input DMAs round-robin over all
three DGE queues (SP + ACT HWDGE, Pool SWDGE) — a single dynamic queue
is serviced by one ~23GB/s DMA engine, which was the baseline kernel's
actual bottleneck.

Per chunk the kernel streams: square (DVE) -> per-column sum-of-squares
via ones-matmul into a [1,2048] PSUM strip (PE) -> inv-norm =
Exp(-0.5*Ln(ssq)) (ACT; the compile patch below pins the one act table
containing BOTH ln and exp so no table reloads occur) ->
partition_broadcast (GpSimd) -> normalize-multiply into fp8e4 (DVE).
Chunk 0 (the lhsT source) is DMA'd column-sliced and normalized in
512-column pieces so the pipeline fills early; its normalized output is
the persistent lhsT.  Main similarity matmuls run in fp8 DoubleRow mode
(K=256 per instruction, 0.5 cycles/row) into [128,2048] PSUM tiles; one
exp activation per (m-tile, chunk) consumes the tile in place with
accum_out producing the partial row-sum.  The next chunk's norm chain is
emitted mid-m-loop so the in-order PE never queues it ahead of main
matmuls.  The between-diagonal is masked off the exp'd own-r PSUM and
log-subtracted in the epilogue.  No cross-core communication.
"""

import numpy as np
import ml_dtypes
from contextlib import ExitStack

import concourse.bass as bass
import concourse.tile as tile
from concourse import bacc, mybir
from concourse.bass import ds, ts
from concourse.bass_utils import run_bass_kernel_spmd
from concourse.masks import make_identity

P = 128          # partitions
D = 512          # feature dim
N = 8192         # rows
NCORES = 8
BLK = N // NCORES          # 1024 rows per core
KCH = D // P               # 4 k-chunks of 128
MT = BLK // P              # 8 m-tiles per core
W = 2048                   # columns per streamed chunk
NJ = N // W                # 4 chunks per tensor
NCHUNK = 2 * NJ            # 8 chunks total (l then r)
NG = W // 512              # 4 psum col-groups per chunk
E2 = float(np.exp(2.0))

F32 = mybir.dt.float32
BF16 = mybir.dt.bfloat16
FP8 = mybir.dt.float8e4
AF = mybir.ActivationFunctionType
OP = mybir.AluOpType
DR = mybir.MatmulPerfMode.DoubleRow

_CACHE = {}


def _body(ctx, tc, lch, rch, loss_out):
    nc = tc.nc

    const_pool = ctx.enter_context(tc.tile_pool(name="const", bufs=1))
    persist = ctx.enter_context(tc.tile_pool(name="persist", bufs=1))
    raw_pool = ctx.enter_context(tc.tile_pool(name="raw", bufs=4))
    sq_pool = ctx.enter_context(tc.tile_pool(name="sq", bufs=2))
    nrm_pool = ctx.enter_context(tc.tile_pool(name="nrm", bufs=2))
    zn_pool = ctx.enter_context(tc.tile_pool(name="zn", bufs=3))
    small = ctx.enter_context(tc.tile_pool(name="small", bufs=2))

    psum = ctx.enter_context(tc.tile_pool(name="ps", bufs=2, space="PSUM"))

    # constants
    ones_col = const_pool.tile([P, 1], BF16, tag="ones_col")
    nc.gpsimd.memset(ones_col[:], 1.0)
    ident = const_pool.tile([P, P], F32, tag="ident")
    make_identity(nc, ident[:])
    neg_e2 = const_pool.tile([P, 1], F32, tag="neg_e2")
    nc.gpsimd.memset(neg_e2[:], -E2)

    # Issue a tiny dummy activation as the very first ACT instruction: the
    # compile pass places the (1.3us) act-table load before it, so the load
    # runs at t~0 instead of stalling the first real Ln of the pipeline.
    warm = const_pool.tile([P, 1], F32, tag="warm")
    nc.scalar.activation(warm[:], neg_e2[:], AF.Exp)

    # persistent tensors
    zn_own = persist.tile([P, KCH, W], FP8, tag="zn_own")   # chunk 0 of l
    rowsums = persist.tile([P, MT, NCHUNK], F32, tag="rowsums")
    bd = persist.tile([P, MT], F32, tag="bd")
    diags = persist.tile([P, MT, P], F32, tag="diags")
    m0x = persist.tile([P, NG - 1], F32, tag="m0x")

    raws = {}
    # Round-robin the input DMAs over all three DGE queues (SP and ACT
    # HWDGE + Pool SWDGE): each dynamic queue is serviced by a single DMA
    # engine at ~23GB/s, so a single queue serializes the whole input
    # stream (this was the baseline's actual bottleneck).
    dma_engines = [nc.sync, nc.scalar, nc.gpsimd]

    def dma_stage(j):
        src = lch[j, :, :, :] if j < NJ else rch[j - NJ, :, :, :]
        raw = raw_pool.tile([P, KCH, W], FP8, tag="raw")
        if j == 0:
            # chunk 0 gates the whole pipeline (it is the lhsT source):
            # column-sliced piece DMAs (512 cols = 512B descriptor lines,
            # the minimum for full DMA-bus efficiency), each split into two
            # k-halves round-robined over the queues, land piece 0 after
            # half a slice time so the piecewise norm chain starts early.
            q = 0
            for g in range(NG):
                for kh in range(2):
                    dma_engines[q % 3].dma_start(
                        raw[:, ds(kh * 2, 2), ds(g * 512, 512)],
                        src[:, ds(kh * 2, 2), ds(g * 512, 512)],
                    )
                    q += 1
        else:
            for k in range(KCH):
                dma_engines[(j * KCH + k) % 3].dma_start(raw[:, k, :], src[:, k, :])
        raws[j] = raw

    zns = {}

    def norm_stage(j, piece_cb=None):
        raw = raws.pop(j)
        sq = sq_pool.tile([P, KCH, W], BF16, tag="sq")
        # chunk 0 is processed in 512-column pieces (its DMA is
        # column-sliced), everything else in one pass
        pieces = NG if j == 0 else 1
        pw = W // pieces
        # Shares the "act" ring slots (same tag) so PSUM stays within 8
        # banks; ring order interleaves [ssq_j+1, act_j_m0..m7, ...].
        ssq = psum.tile([1, W], F32, tag="act")
        lssq = nrm_pool.tile([1, W], F32, tag="lssq")
        inv1 = nrm_pool.tile([1, W], BF16, tag="inv1")
        invb = nrm_pool.tile([P, W], BF16, tag="invb")
        zn = zn_own if j == 0 else zn_pool.tile([P, KCH, W], FP8, tag="zn")
        for p_ in range(pieces):
            cs = ds(p_ * pw, pw)
            nc.vector.tensor_mul(sq[:, :, cs], raw[:, :, cs], raw[:, :, cs])
            for g in range(pw // 512):
                c0 = p_ * pw + g * 512
                for k in range(KCH):
                    nc.tensor.matmul(
                        ssq[:, ds(c0, 512)],
                        ones_col[:],
                        sq[:, k, ds(c0, 512)],
                        start=(k == 0),
                        stop=(k == KCH - 1),
                    )
            nc.scalar.activation(lssq[:, cs], ssq[:, cs], AF.Ln)
            nc.scalar.activation(inv1[:, cs], lssq[:, cs], AF.Exp, scale=-0.5)
            nc.gpsimd.partition_broadcast(invb[:, cs], inv1[:, cs])
            for k in range(KCH):
                nc.vector.tensor_mul(zn[:, k, cs], raw[:, k, cs], invb[:, cs])
            if piece_cb is not None:
                piece_cb(p_)
        zns[j] = zn

    def main_stage(j, m_start=0):
        zn = zns.pop(j)
        for m in range(m_start, MT):
            ps = psum.tile([P, W], F32, tag="act")
            for g in range(NG):
                for i in range(KCH // 2):
                    nc.tensor.matmul(
                        ps[:, ds(g * 512, 512)],
                        zn_own[:, ds(2 * i, 2), ts(m, P)],
                        zn[:, ds(2 * i, 2), ds(g * 512, 512)],
                        start=(i == 0),
                        stop=(i == KCH // 2 - 1),
                        perf_mode=DR,
                    )
            if j == NJ:
                # own-r chunk: snapshot the raw diagonal block on ACT itself
                # (cheap Copy) right before the in-place exp.  Anything else
                # reading the PSUM here (e.g. DVE) would hold the ring slot
                # hostage behind the next chunk's norm work.
                nc.scalar.activation(
                    diags[:, m, :], ps[:, ds(m * P, P)], AF.Copy
                )
            # exp output is discarded (only accum_out matters): write back
            # into the same PSUM tile (PSUM access is cheaper for ACT than
            # SBUF, and no SBUF bandwidth is spent on a dummy tensor).
            nc.scalar.activation(
                ps[:], ps[:], AF.Exp, scale=2.0,
                accum_out=rowsums[:, m, ds(j, 1)],
            )
            # software pipeline, two chunks ahead: norm chains are emitted
            # mid-m-loop (never ahead of main matmuls on the in-order PE),
            # and chunk j+2 is normalized during main_stage(j) so at each
            # chunk boundary the next zn is already resident -- the
            # broadcast+multiply+matmul latency never shows on ACT.
            if m == 1:
                if j == 0:
                    norm_stage(1)
                    dma_stage(3)
                elif j + 2 < NCHUNK:
                    norm_stage(j + 2)
                    if j + 4 < NCHUNK:
                        dma_stage(j + 4)
            if m == 5 and j == 0:
                norm_stage(2)
                dma_stage(4)

    dma_stage(0)
    dma_stage(1)
    dma_stage(2)

    # Fused prologue: as each 512-col piece of chunk 0 is normalized, emit
    # m-tile 0's matmul group and piece-exp for it immediately, so the
    # in-order PE never sits behind the remaining ssq pieces and the first
    # activation fires ~one piece after the first DMA lands.
    ps_m0 = [None]

    def m0_piece(g):
        if ps_m0[0] is None:
            ps_m0[0] = psum.tile([P, W], F32, tag="act", name="ps_m0")
        ps = ps_m0[0]
        for i in range(KCH // 2):
            nc.tensor.matmul(
                ps[:, ds(g * 512, 512)],
                zn_own[:, ds(2 * i, 2), ts(0, P)],
                zn_own[:, ds(2 * i, 2), ds(g * 512, 512)],
                start=(i == 0),
                stop=(i == KCH // 2 - 1),
                perf_mode=DR,
            )
        nc.scalar.activation(
            ps[:, ds(g * 512, 512)], ps[:, ds(g * 512, 512)],
            AF.Exp, scale=2.0,
            accum_out=rowsums[:, 0, ds(0, 1)] if g == 0
            else m0x[:, ds(g - 1, 1)],
        )

    norm_stage(0, piece_cb=m0_piece)
    main_stage(0, m_start=1)
    for j in range(1, NCHUNK):
        main_stage(j)
        if j == NJ:
            # diags is complete once the own-r chunk is done; mask+reduce
            # now so it rides in DVE idle time instead of the drain tail
            for m in range(MT):
                dtmp = small.tile([P, P], F32, tag="dtmp")
                nc.vector.tensor_mul(dtmp[:], diags[:, m, :], ident[:])
                nc.vector.tensor_reduce(
                    bd[:, ts(m, 1)], dtmp[:],
                    axis=mybir.AxisListType.X, op=OP.add,
                )

    # ---- loss epilogue -----------------------------------------------------
    # loss = Ln(S_l + S_r - e^2) - Ln(exp(2*zl.zr))  [bd holds the exp'd diag]
    s_all = small.tile([P, MT], F32, tag="s_all")
    nc.vector.tensor_reduce(
        s_all[:], rowsums[:], axis=mybir.AxisListType.X, op=OP.add
    )
    m0s = small.tile([P, 1], F32, tag="m0s")
    nc.vector.tensor_reduce(
        m0s[:], m0x[:], axis=mybir.AxisListType.X, op=OP.add
    )
    nc.vector.tensor_add(s_all[:, ds(0, 1)], s_all[:, ds(0, 1)], m0s[:])
    logd = small.tile([P, MT], F32, tag="logd")
    nc.scalar.activation(logd[:], s_all[:], AF.Ln, bias=neg_e2[:])
    loss_sb = small.tile([P, MT], F32, tag="loss_sb")
    nc.vector.scalar_tensor_tensor(
        out=loss_sb[:], in0=bd[:], scalar=-2.0, in1=logd[:],
        op0=OP.mult, op1=OP.add,
    )
    nc.sync.dma_start(loss_out[:, :], loss_sb[:])


class _pin_act_table:
    """During compile, present activation tables where Exp/Ln appear ONLY in
    the combined natural_log_exp table, so the table-load pass emits a single
    hoisted load instead of ping-ponging between an exp-table and an
    ln-table (1.3us per switch).  Table order/indices are preserved; only
    the membership sets are filtered.  Restored immediately after compile."""

    COMBINED = "natural_log_exp_and_others"

    def __enter__(self):
        import concourse.bacc as bacc_mod
        self._mod = bacc_mod
        self._orig = bacc_mod.get_activation_tables

        orig = self._orig
        combined = self.COMBINED

        def patched(arch):
            tabs = orig(arch)
            if combined not in tabs:
                return tabs
            pin = {AF.Exp, AF.Ln}
            out = {}
            for name, s in tabs.items():
                out[name] = set(s) if name == combined else set(s) - pin
            return out

        bacc_mod.get_activation_tables = patched
        return self

    def __exit__(self, *exc):
        self._mod.get_activation_tables = self._orig
        return False


def _build():
    nc = bacc.Bacc("TRN2", target_bir_lowering=False, debug=False, num_devices=NCORES)
    lch = nc.dram_tensor("lch", [NJ, P, KCH, W], FP8, kind="ExternalInput").ap()
    rch = nc.dram_tensor("rch", [NJ, P, KCH, W], FP8, kind="ExternalInput").ap()
    loss = nc.dram_tensor("loss", [P, MT], F32, kind="ExternalOutput").ap()
    with tile.TileContext(nc) as tc, ExitStack() as ctx:
        _body(ctx, tc, lch, rch, loss)
    with _pin_act_table():
        nc.compile()
    return nc


def _get_nc():
    if "nc" not in _CACHE:
        _CACHE["nc"] = _build()
    return _CACHE["nc"]


def _chunked(xT, c):
    """xT: [KCH, P, N] bf16 K-major. Returns [NJ, P, KCH, W] rolled so core
    c's own columns come first."""
    r = np.roll(xT, -c * BLK, axis=2)
    # [KCH, P, N] -> [NJ, P, KCH, W]
    out = np.empty((NJ, P, KCH, W), dtype=xT.dtype)
    for j in range(NJ):
        out[j] = r[:, :, j * W:(j + 1) * W].transpose(1, 0, 2)
    return out


def _in_maps(left, right):
    f8 = ml_dtypes.float8_e4m3
    left = np.asarray(left, dtype=np.float32)
    right = np.asarray(right, dtype=np.float32)
    lT = np.ascontiguousarray(left.T).astype(f8).reshape(KCH, P, N)
    rT = np.ascontiguousarray(right.T).astype(f8).reshape(KCH, P, N)
    maps = []
    for c in range(NCORES):
        maps.append({
            "lch": _chunked(lT, c),
            "rch": _chunked(rT, c),
        })
    return maps


def _gather(results):
    # loss dram tile is [128 partitions, 8 m-tiles]; row m = t*128 + p
    parts = [np.asarray(r["loss"]).T.reshape(-1) for r in results]
    return np.concatenate(parts).astype(np.float32)


def run_traced(left, right):
    """Run with NTFF profiling; returns (loss, exec_time_ns)."""
    res = run_bass_kernel_spmd(
        _get_nc(), _in_maps(left, right), list(range(NCORES)), trace=True
    )
    return _gather(res.results), res.exec_time_ns


def kernel(left, right):
    res = run_bass_kernel_spmd(
        _get_nc(), _in_maps(left, right), list(range(NCORES))
    )
    return _gather(res.results)
